# revision 5
# baseline (speedup 1.0000x reference)
"""Batched full self-convolution on 8 Trainium2 NeuronCores.

out[b] = conv(x[b], x[b], mode='full');  x: [16384, 512] f32 -> out: [16384, 1023].

Algorithm: out = IRFFT(RFFT(pad(x, 1024))^2) realized as two dense matmuls with
fixed (batch-uniform) packed-DFT matrices, so all the FLOPs land on the PE
systolic array with batch as the free dimension:

  packed fwd:  P = x @ C        C: [512, 1024]   P = [R[0..512] | I[1..511]]
  squares:     Z = pack(R^2 - I^2, R*I)      (elementwise, DVE)
  packed inv:  out = Z @ D      D: [1024, 1024]  (col 1023 is padding)

Matmuls run in float32r (fp32 with 11-bit mantissa, 1 cycle/row on PE at
free-dim 512 — same speed as bf16, ~13x better accuracy). Data parallel over
batch: 2048 rows per core, no cross-core communication.
"""
import sys

sys.path.insert(0, '/opt/trn_rl_repo')

import numpy as np

import concourse.bass as bass
import concourse.tile as tile
from concourse import mybir
from concourse.bass_utils import run_bass_kernel_spmd
from concourse.masks import make_identity

F32 = mybir.dt.float32
F32R = mybir.dt.float32r

BATCH = 16384
N = 512            # input length per row
M = 1024           # DFT size
NT = 1023          # output length per row
N_CORES = 8
R = BATCH // N_CORES   # rows per core = 2048
RT = R // 128          # row tiles per core = 16
KC = N // 128          # contraction chunks over n = 4
FC = M // 128          # frequency chunks = 8


def _rn12(a: np.ndarray) -> np.ndarray:
    """Round fp32 to float32r's storage precision (11-bit mantissa)."""
    u = a.astype(np.float32).view(np.uint32).astype(np.uint64)
    u = ((u + 0x800) & 0xFFFFF000).astype(np.uint32)
    return u.view(np.float32)


def _dft_matrices():
    n = np.arange(N, dtype=np.float64)[:, None]
    f = np.arange(M // 2 + 1, dtype=np.float64)[None, :]
    ang = -2.0 * np.pi * n * f / M
    # packed forward: cols 0..512 = Re, cols 513..1023 = Im[1..511]
    C = np.concatenate([np.cos(ang), np.sin(ang)[:, 1:512]], axis=1)

    t = np.arange(M, dtype=np.float64)[None, :]  # col 1023 = padding
    D = np.zeros((M, M), dtype=np.float64)
    D[0, :] = 1.0 / M
    D[512, :] = ((-1.0) ** t[0]) / M
    ff = np.arange(1, 512, dtype=np.float64)[:, None]
    D[1:512, :] = 2.0 * np.cos(2.0 * np.pi * ff * t / M) / M
    # Z's Im part stores R*I (factor 2 folded here: -2 * 2 = -4)
    D[513:, :] = -4.0 * np.sin(2.0 * np.pi * ff * t / M) / M
    D[:, NT:] = 0.0
    return _rn12(C), _rn12(D)


def _build_nc():
    nc = bass.Bass("TRN2", target_bir_lowering=False, debug=False,
                   num_devices=N_CORES)
    x_d = nc.dram_tensor("x", [R, N], F32, kind="ExternalInput").ap()
    c_d = nc.dram_tensor("cmat", [N, M], F32R, kind="ExternalInput").ap()
    d_d = nc.dram_tensor("dmat", [M, M], F32R, kind="ExternalInput").ap()
    o_d = nc.dram_tensor("out", [R, NT], F32, kind="ExternalOutput").ap()

    with tile.TileContext(nc) as tc:
        with tc.tile_pool(name="const", bufs=1) as const_pool, \
             tc.tile_pool(name="xin", bufs=4) as xin_pool, \
             tc.tile_pool(name="xt", bufs=1) as xt_pool, \
             tc.tile_pool(name="fz", bufs=1) as fz_pool, \
             tc.tile_pool(name="tmp", bufs=2) as tmp_pool, \
             tc.tile_pool(name="osb", bufs=3) as osb_pool, \
             tc.tile_pool(name="tp_ps", bufs=2, space="PSUM") as tp_ps, \
             tc.tile_pool(name="s1_ps", bufs=2, space="PSUM") as s1_ps, \
             tc.tile_pool(name="s2_ps", bufs=4, space="PSUM") as s2_ps:

            # ---- constants ----
            ident = const_pool.tile([128, 128], F32, name="ident")
            make_identity(nc, ident[:])
            c_sb = [const_pool.tile([128, M], F32R, name=f"c_sb{k}")
                    for k in range(KC)]
            for k in range(KC):
                nc.sync.dma_start(c_sb[k][:], c_d[128 * k:128 * (k + 1), :])
            d_sb = [const_pool.tile([128, M], F32R, name=f"d_sb{k}")
                    for k in range(FC)]
            for k in range(FC):
                nc.sync.dma_start(d_sb[k][:], d_d[128 * k:128 * (k + 1), :])

            # ---- load x and transpose into XT[k][n within chunk, row] ----
            xt_sb = [xt_pool.tile([128, R], F32R, name=f"xt{k}")
                     for k in range(KC)]
            for bt in range(RT):
                x_tile = xin_pool.tile([128, N], F32, tag="x_tile")
                nc.sync.dma_start(x_tile[:], x_d[128 * bt:128 * (bt + 1), :])
                for k in range(KC):
                    ps = tp_ps.tile([128, 128], F32, tag="tp")
                    nc.tensor.transpose(ps[:], x_tile[:, 128 * k:128 * (k + 1)],
                                        ident[:])
                    # copyback rounds to f32r (producer-side rounding)
                    nc.vector.tensor_copy(
                        xt_sb[k][:, 128 * bt:128 * (bt + 1)], ps[:])

            # ---- stage 1: F^T[f, b] = sum_n C[n, f] * XT[n, b] ----
            # tiles typed f32r so every writer is a producer-side rounder
            # (the FP32r verifier checks all writers of matmul inputs);
            # f32r stores plain rounded-fp32 bits, so DVE reads them as f32.
            fz_sb = [fz_pool.tile([128, R], F32R, name=f"fz_sb{j}")
                     for j in range(FC)]
            f_sb = [t.bitcast(F32) for t in fz_sb]
            for ft in range(FC):
                for bc in range(R // 512):
                    ps = s1_ps.tile([128, 512], F32, tag="s1")
                    for k in range(KC):
                        nc.tensor.matmul(
                            ps[:],
                            lhsT=c_sb[k][:, 128 * ft:128 * (ft + 1)],
                            rhs=xt_sb[k][:, 512 * bc:512 * (bc + 1)],
                            start=(k == 0), stop=(k == KC - 1))
                    nc.vector.tensor_copy(fz_sb[ft][:, 512 * bc:512 * (bc + 1)],
                                          ps[:])

            # ---- squares (in place: Z overwrites F, viewed as f32r) ----
            # pair j: R rows in f_sb[j], I rows in f_sb[4+j] (packed offset 512)
            z_sb = fz_sb
            for j in range(FC // 2):
                rj, ij = f_sb[j], f_sb[4 + j]
                t1 = tmp_pool.tile([128, R], F32, tag="t1")
                t2 = tmp_pool.tile([128, R], F32, tag="t2")
                nc.vector.tensor_mul(t1[:], rj[:], rj[:])
                nc.vector.tensor_mul(t2[:], ij[:], ij[:])
                # RI must read original rj before it is overwritten
                nc.vector.tensor_mul(z_sb[4 + j][:], rj[:], ij[:])
                nc.vector.tensor_sub(z_sb[j][:], t1[:], t2[:])
                if j == 0:
                    # partition 0 of pair 0 holds (R[0], R[512]), not (R, I):
                    # packed Z[0] must be R[0]^2 = t1[0], Z[512] must be
                    # R[512]^2 = t2[0].
                    nc.vector.tensor_copy(z_sb[0][0:1, :], t1[0:1, :])
                    nc.vector.tensor_copy(z_sb[4][0:1, :], t2[0:1, :])

            # ---- stage 2: out[b, t] = sum_f Z[f, b] * D[f, t] ----
            for bt in range(RT):
                o_tile = osb_pool.tile([128, M], F32, tag="o_tile")
                for tg in range(2):
                    ps = s2_ps.tile([128, 512], F32, tag="s2")
                    for k in range(FC):
                        nc.tensor.matmul(
                            ps[:],
                            lhsT=z_sb[k][:, 128 * bt:128 * (bt + 1)],
                            rhs=d_sb[k][:, 512 * tg:512 * (tg + 1)],
                            start=(k == 0), stop=(k == FC - 1))
                    nc.vector.tensor_copy(o_tile[:, 512 * tg:512 * (tg + 1)],
                                          ps[:])
                nc.sync.dma_start(o_d[128 * bt:128 * (bt + 1), :],
                                  o_tile[:, :NT])

    _split_multi_waits(nc)
    return nc


def _split_multi_waits(nc):
    """This walrus build rejects instructions carrying more than one sync-wait
    command. Hoist all but the last wait of each instruction onto standalone
    single-wait InstEventSemaphore ops inserted just before it (same engine),
    preserving Tile's synchronization exactly."""
    wid = 0
    for fn in nc.m.functions:
        for bb in fn.blocks:
            out = []
            changed = False
            for inst in bb.instructions:
                si = inst.sync_info
                if si is not None and si.on_wait and len(si.on_wait) > 1:
                    waits = list(si.on_wait)
                    for w in waits[:-1]:
                        wid += 1
                        wi = mybir.InstEventSemaphore(
                            name=f"WS-{wid}", ins=[], outs=[])
                        wi.engine = inst.engine
                        wi.sync_info = mybir.SyncInfo(on_wait=[w], on_update=[])
                        out.append(wi)
                    inst.sync_info = mybir.SyncInfo(
                        on_wait=[waits[-1]], on_update=list(si.on_update or []))
                    changed = True
                out.append(inst)
            if changed:
                bb.instructions = out


_CACHE = {}


def _get_nc():
    if "nc" not in _CACHE:
        _CACHE["nc"] = _build_nc()
        _CACHE["consts"] = _dft_matrices()
    return _CACHE["nc"], _CACHE["consts"]


def kernel(x_0: np.ndarray) -> np.ndarray:
    assert x_0.shape == (BATCH, N) and x_0.dtype == np.float32
    nc, (C, D) = _get_nc()
    in_maps = [
        {"x": np.ascontiguousarray(x_0[R * c:R * (c + 1)]),
         "cmat": C, "dmat": D}
        for c in range(N_CORES)
    ]
    res = run_bass_kernel_spmd(nc, in_maps, list(range(N_CORES)))
    return np.concatenate([res.results[c]["out"] for c in range(N_CORES)],
                          axis=0)


if __name__ == "__main__":
    rng = np.random.default_rng(0)
    x = rng.standard_normal((BATCH, N)).astype(np.float32)
    out = kernel(x)
    print("out", out.shape, out.dtype)


# revision 6
# speedup vs baseline: 1647.9772x; 1647.9772x over previous
"""Batched full self-convolution on 8 Trainium2 NeuronCores.

out[b] = conv(x[b], x[b], mode='full');  x: [16384, 512] f32 -> out: [16384, 1023].

Algorithm: out = IRFFT(RFFT(pad(x, 1024))^2) realized as two dense matmuls with
fixed (batch-uniform) packed-DFT matrices, so all the FLOPs land on the PE
systolic array with batch as the free dimension:

  packed fwd:  P = x @ C        C: [512, 1024]   P = [R[0..512] | I[1..511]]
  squares:     Z = pack(R^2 - I^2, R*I)      (elementwise, DVE)
  packed inv:  out = Z @ D      D: [1024, 1024]  (col 1023 is padding)

Matmuls run in float32r (fp32 with 11-bit mantissa, 1 cycle/row on PE at
free-dim 512 — same speed as bf16, ~13x better accuracy). Data parallel over
batch: 2048 rows per core, no cross-core communication.
"""
import sys

sys.path.insert(0, '/opt/trn_rl_repo')

import numpy as np

import concourse.bass as bass
import concourse.tile as tile
from concourse import mybir
from concourse.bass_utils import run_bass_kernel_spmd
from concourse.masks import make_identity

F32 = mybir.dt.float32
F32R = mybir.dt.float32r

BATCH = 16384
N = 512            # input length per row
M = 1024           # DFT size
NT = 1023          # output length per row
N_CORES = 8
R = BATCH // N_CORES   # rows per core = 2048
RT = R // 128          # row tiles per core = 16
KC = N // 128          # contraction chunks over n = 4
FC = M // 128          # frequency chunks = 8


def _rn12(a: np.ndarray) -> np.ndarray:
    """Round fp32 to float32r's storage precision (11-bit mantissa)."""
    u = a.astype(np.float32).view(np.uint32).astype(np.uint64)
    u = ((u + 0x800) & 0xFFFFF000).astype(np.uint32)
    return u.view(np.float32)


def _dft_matrices():
    n = np.arange(N, dtype=np.float64)[:, None]
    f = np.arange(M // 2 + 1, dtype=np.float64)[None, :]
    ang = -2.0 * np.pi * n * f / M
    # packed forward: cols 0..512 = Re, cols 513..1023 = Im[1..511]
    C = np.concatenate([np.cos(ang), np.sin(ang)[:, 1:512]], axis=1)

    t = np.arange(M, dtype=np.float64)[None, :]  # col 1023 = padding
    D = np.zeros((M, M), dtype=np.float64)
    D[0, :] = 1.0 / M
    D[512, :] = ((-1.0) ** t[0]) / M
    ff = np.arange(1, 512, dtype=np.float64)[:, None]
    D[1:512, :] = 2.0 * np.cos(2.0 * np.pi * ff * t / M) / M
    # Z's Im part stores R*I (factor 2 folded here: -2 * 2 = -4)
    D[513:, :] = -4.0 * np.sin(2.0 * np.pi * ff * t / M) / M
    D[:, NT:] = 0.0
    return _rn12(C), _rn12(D)


def _build_nc(reps: int = 1):
    nc = bass.Bass("TRN2", target_bir_lowering=False, debug=False,
                   num_devices=N_CORES)
    x_d = nc.dram_tensor("x", [R, N], F32, kind="ExternalInput").ap()
    c_d = nc.dram_tensor("cmat", [N, M], F32R, kind="ExternalInput").ap()
    d_d = nc.dram_tensor("dmat", [M, M], F32R, kind="ExternalInput").ap()
    o_d = nc.dram_tensor("out", [R, NT], F32, kind="ExternalOutput").ap()

    with tile.TileContext(nc) as tc:
        with tc.tile_pool(name="const", bufs=1) as const_pool, \
             tc.tile_pool(name="xin", bufs=4) as xin_pool, \
             tc.tile_pool(name="xt", bufs=1) as xt_pool, \
             tc.tile_pool(name="fz", bufs=1) as fz_pool, \
             tc.tile_pool(name="tmp", bufs=2) as tmp_pool, \
             tc.tile_pool(name="osb", bufs=3) as osb_pool, \
             tc.tile_pool(name="tp_ps", bufs=2, space="PSUM") as tp_ps, \
             tc.tile_pool(name="s1_ps", bufs=2, space="PSUM") as s1_ps, \
             tc.tile_pool(name="s2_ps", bufs=4, space="PSUM") as s2_ps:

            # ---- constants ----
            ident = const_pool.tile([128, 128], F32, name="ident")
            make_identity(nc, ident[:])
            c_sb = [const_pool.tile([128, M], F32R, name=f"c_sb{k}")
                    for k in range(KC)]
            for k in range(KC):
                nc.sync.dma_start(c_sb[k][:], c_d[128 * k:128 * (k + 1), :])
            d_sb = [const_pool.tile([128, M], F32R, name=f"d_sb{k}")
                    for k in range(FC)]
            for k in range(FC):
                nc.sync.dma_start(d_sb[k][:], d_d[128 * k:128 * (k + 1), :])

            xt_sb = [xt_pool.tile([128, R], F32R, name=f"xt{k}")
                     for k in range(KC)]
            # tiles typed f32r so every writer is a producer-side rounder
            # (the FP32r verifier checks all writers of matmul inputs);
            # f32r stores plain rounded-fp32 bits, so DVE reads them as f32.
            fz_sb = [fz_pool.tile([128, R], F32R, name=f"fz_sb{j}")
                     for j in range(FC)]
            f_sb = [t.bitcast(F32) for t in fz_sb]
            z_sb = fz_sb

            for _rep in range(reps):
                # ---- load x, transpose into XT[k][n within chunk, row] ----
                for bt in range(RT):
                    x_tile = xin_pool.tile([128, N], F32, tag="x_tile",
                                           name="x_tile")
                    nc.sync.dma_start(x_tile[:],
                                      x_d[128 * bt:128 * (bt + 1), :])
                    for k in range(KC):
                        ps = tp_ps.tile([128, 128], F32, tag="tp", name="tp")
                        nc.tensor.transpose(
                            ps[:], x_tile[:, 128 * k:128 * (k + 1)], ident[:])
                        # copyback rounds to f32r (producer-side rounding)
                        nc.vector.tensor_copy(
                            xt_sb[k][:, 128 * bt:128 * (bt + 1)], ps[:])

                # ---- stage 1: F^T[f, b] = sum_n C[n, f] * XT[n, b] ----
                for ft in range(FC):
                    for bc in range(R // 512):
                        ps = s1_ps.tile([128, 512], F32, tag="s1", name="s1")
                        for k in range(KC):
                            nc.tensor.matmul(
                                ps[:],
                                lhsT=c_sb[k][:, 128 * ft:128 * (ft + 1)],
                                rhs=xt_sb[k][:, 512 * bc:512 * (bc + 1)],
                                start=(k == 0), stop=(k == KC - 1))
                        nc.vector.tensor_copy(
                            fz_sb[ft][:, 512 * bc:512 * (bc + 1)], ps[:])

                # ---- squares (in place: Z overwrites F, viewed as f32r);
                # pair j: R rows in f_sb[j], I rows in f_sb[4+j] ----
                for j in range(FC // 2):
                    rj, ij = f_sb[j], f_sb[4 + j]
                    t1 = tmp_pool.tile([128, R], F32, tag="t1", name="t1")
                    t2 = tmp_pool.tile([128, R], F32, tag="t2", name="t2")
                    nc.vector.tensor_mul(t1[:], rj[:], rj[:])
                    nc.vector.tensor_mul(t2[:], ij[:], ij[:])
                    # RI must read original rj before it is overwritten
                    nc.vector.tensor_mul(z_sb[4 + j][:], rj[:], ij[:])
                    nc.vector.tensor_sub(z_sb[j][:], t1[:], t2[:])
                    if j == 0:
                        # partition 0 of pair 0 holds (R[0], R[512]), not
                        # (R, I): packed Z[0] must be R[0]^2 = t1[0],
                        # Z[512] must be R[512]^2 = t2[0].
                        nc.vector.tensor_copy(z_sb[0][0:1, :], t1[0:1, :])
                        nc.vector.tensor_copy(z_sb[4][0:1, :], t2[0:1, :])

                # ---- stage 2: out[b, t] = sum_f Z[f, b] * D[f, t] ----
                for bt in range(RT):
                    o_tile = osb_pool.tile([128, M], F32, tag="o_tile",
                                           name="o_tile")
                    for tg in range(2):
                        ps = s2_ps.tile([128, 512], F32, tag="s2", name="s2")
                        for k in range(FC):
                            nc.tensor.matmul(
                                ps[:],
                                lhsT=z_sb[k][:, 128 * bt:128 * (bt + 1)],
                                rhs=d_sb[k][:, 512 * tg:512 * (tg + 1)],
                                start=(k == 0), stop=(k == FC - 1))
                        nc.vector.tensor_copy(
                            o_tile[:, 512 * tg:512 * (tg + 1)], ps[:])
                    nc.sync.dma_start(o_d[128 * bt:128 * (bt + 1), :],
                                      o_tile[:, :NT])

    _split_multi_waits(nc)
    return nc


def _split_multi_waits(nc):
    """This walrus build rejects instructions carrying more than one sync-wait
    command. Hoist all but the last wait of each instruction onto standalone
    single-wait InstEventSemaphore ops inserted just before it (same engine),
    preserving Tile's synchronization exactly."""
    wid = 0
    for fn in nc.m.functions:
        for bb in fn.blocks:
            out = []
            changed = False
            for inst in bb.instructions:
                si = inst.sync_info
                if si is not None and si.on_wait and len(si.on_wait) > 1:
                    waits = list(si.on_wait)
                    for w in waits[:-1]:
                        wid += 1
                        wi = mybir.InstEventSemaphore(
                            name=f"WS-{wid}", ins=[], outs=[])
                        wi.engine = inst.engine
                        wi.sync_info = mybir.SyncInfo(on_wait=[w], on_update=[])
                        out.append(wi)
                    inst.sync_info = mybir.SyncInfo(
                        on_wait=[waits[-1]], on_update=list(si.on_update or []))
                    changed = True
                out.append(inst)
            if changed:
                bb.instructions = out


_CACHE = {}


def _get_nc():
    if "nc" not in _CACHE:
        _CACHE["nc"] = _build_nc()
        _CACHE["consts"] = _dft_matrices()
    return _CACHE["nc"], _CACHE["consts"]


def kernel(x_0: np.ndarray) -> np.ndarray:
    assert x_0.shape == (BATCH, N) and x_0.dtype == np.float32
    nc, (C, D) = _get_nc()
    in_maps = [
        {"x": np.ascontiguousarray(x_0[R * c:R * (c + 1)]),
         "cmat": C, "dmat": D}
        for c in range(N_CORES)
    ]
    res = run_bass_kernel_spmd(nc, in_maps, list(range(N_CORES)))
    return np.concatenate([res.results[c]["out"] for c in range(N_CORES)],
                          axis=0)


if __name__ == "__main__":
    rng = np.random.default_rng(0)
    x = rng.standard_normal((BATCH, N)).astype(np.float32)
    out = kernel(x)
    print("out", out.shape, out.dtype)
